# revision 21
# baseline (speedup 1.0000x reference)
"""BertAttention (QKV proj + MHA + output proj + residual + LayerNorm) on 8 TRN2 NeuronCores.

Sharding: batch (4-way) x query-sequence-half (2-way) => 8 shards, no collectives.
Core c handles batch b=c//2, query half c%2. Each core computes K/V for its full
batch sequence (all heads) and Q/attention/output-proj/LayerNorm for its 1024
query rows. K/V projection work is duplicated across the 2 cores sharing a batch;
in exchange there is zero cross-core communication.

The host permutes each core's X rows so its query half comes first — attention is
permutation-invariant over keys as long as (K, V, mask) share the permutation, so
the program is identical across cores (pure SPMD) with no per-core indices.

Host pre-stages inputs: X transposed to [H, S] fp8 (feature on partitions after
DMA), weights cast to fp8, residual rows kept fp32. This removes all on-device
casts and PE transposes and cuts the load DMA bytes.

Layouts (SBUF partition dim first):
  xt:      [128, H/128, S]   transposed activations, fp8 (direct DMA)
  Kt:      [128, H/128, S]   transposed keys (feature on partitions), bf16
  Qt:      [128, H/128, SH]  transposed, bf16
  V:       [128, S/128, NH*65] natural ([tok, head-dim]) with a ones column per
           head at slot 64 — the PV matmul then yields sum(exp) as row 64 for free
  scores:  St[ktok, qtok] in PSUM; softmax sum over ktok (the partition dim) comes
           from the ones-column trick; max-subtraction safely skipped (|s| <~ 1)
  ctx:     [128, NH/2, SH]   transposed (head dim on partitions), fp8
  out:     natural [qtok, H] — residual add + LayerNorm along the free dim.

Attention runs over HEAD PAIRS (2j, 2j+1): head 2j lives on partitions 0-63 of
its Kt/Qt tile, head 2j+1 on partitions 64-127, so the two scores matmuls are
64-contraction tiles at PE row positions 0 and 64 — the PE runs them
CONCURRENTLY (tile concurrency), doubling effective scores throughput vs the
half-array single-head form. PSUM: 2 rotating [128,SH] score tiles (4 banks) +
2 per-pair PV accumulators [65,SH] (4 banks) = all 8 banks.

The softmax exp (the largest non-PE cost: 256 [128,1024] tiles/core) is split
between ACT (native Exp) and DVE (Schraudolph fast-exp:
bitcast_f32(i32(y*A+B)) via tensor_scalar + bitcast copy) to balance the two
engine walls. Projection work (K/Q/V + late Wo) is deadline-paced into the
pair loop as PE gap-filler, borrowing score-PSUM rotation slots.
"""

from contextlib import ExitStack

import numpy as np
import ml_dtypes

import bass_rust
import concourse.bass as bass
import concourse.mybir as mybir
from concourse.tile import TileContext
from concourse.bass_utils import run_bass_kernel_spmd

FP = mybir.dt.float32
BF = mybir.dt.bfloat16
E4 = mybir.dt.float8e4
DR = mybir.MatmulPerfMode.DoubleRow
AF = mybir.ActivationFunctionType
OP = mybir.AluOpType

N_CORES = 8
EPS = 1e-12

# e4m3-domain Schraudolph fast-exp: e4m3_bits(exp(y)) ~ int8(y*8/ln2 + B8),
# one DVE tensor_scalar (float->int8 convert) writing through a bitcast view
# of the fp8 p tile. Host-swept constant: max elem rel err 7.3% (the fp32
# Schraudolph + fp8 cast it replaces measured 8.0%), cancelled by softmax
# normalization. (GpSimd measured ~4.3us per [128,1024] converting cast —
# not viable for bulk elementwise work, so exp runs on ACT and DVE only.)
FE8_A = 8.0 / float(np.log(2.0))
FE8_B = 55.63

# The walrus build in this toolchain rejects instructions that carry more than
# one sync-wait command ("Too many sync wait commands", CoreV2/V3 setupSyncWait),
# while Tile freely attaches several semaphore waits to one instruction (and the
# TileContext exit drain aggregates one wait per logical processor). Hoist the
# excess waits onto standalone InstEventSemaphore carriers on the same engine,
# placed immediately before the instruction — engine streams are serial, so the
# gating semantics are identical.
_MAX_WAITS_PER_INST = 1


def _split_sync_waits(nc, cap=_MAX_WAITS_PER_INST):
    n_split = 0
    for fn in nc.m.functions:
        for bb in fn.blocks:
            insts = list(bb.instructions)
            out = []
            changed = False
            for ins in insts:
                si = ins.sync_info
                waits = list(si.on_wait) if (si is not None and si.on_wait) else []
                if len(waits) > cap:
                    head, tail = waits[: len(waits) - cap], waits[len(waits) - cap :]
                    for j, w in enumerate(head):
                        ev = mybir.InstEventSemaphore(
                            name=f"{ins.name}-sw{j}",
                            engine=ins.engine,
                            ins=[],
                            outs=[],
                            sync_info=bass_rust.SyncInfo(on_wait=[w], on_update=[]),
                        )
                        out.append(ev)
                        n_split += 1
                    si.on_wait = tail
                    changed = True
                out.append(ins)
            if changed:
                bb.instructions[:] = out
    return n_split


def _dram_row_bcast(handle, p, n):
    """AP reading DRAM vector [n] broadcast across p partitions."""
    return bass.AP(tensor=handle, offset=0, ap=[[0, p], [1, n]])


def _build(s, h, nh, sh, flags):
    """Build the per-core Bass program. flags: which bias/affine inputs matter."""
    hd = h // nh
    assert hd == 64, "head packing assumes head_dim 64 (2 heads per 128 partitions)"
    kt_n = h // 128  # contraction tiles over hidden dim
    tt_n = s // 128  # key-token tiles
    qt_n = sh // 128  # query-token tiles
    scale = 1.0 / float(np.sqrt(hd))

    nc = bass.Bass(target_bir_lowering=False)
    x = nc.dram_tensor("x", [sh, h], FP, kind="ExternalInput")  # residual rows
    xt_d = nc.dram_tensor("xt", [h, s], E4, kind="ExternalInput")  # transposed
    mask = nc.dram_tensor("mask", [s], FP, kind="ExternalInput")
    w_dram = {
        n: nc.dram_tensor(n, [h, h], E4, kind="ExternalInput")
        for n in ("wq", "wk", "wv", "wo")
    }
    vec_dram = {
        n: nc.dram_tensor(n, [h], FP, kind="ExternalInput")
        for n in ("bq", "bk", "bv", "bo", "ln_gamma", "ln_beta")
        if flags[n]
    }
    out = nc.dram_tensor("out", [sh, h], FP, kind="ExternalOutput")

    with TileContext(nc) as tc, ExitStack() as st_all:
        persist = st_all.enter_context(tc.tile_pool(name="persist", bufs=1))
        dram = st_all.enter_context(tc.tile_pool(name="dram", bufs=1, space="DRAM"))
        qt = persist.tile([128, kt_n, sh], BF)
        kt = persist.tile([128, kt_n, s], BF)
        vsb = persist.tile([128, tt_n, nh * 65], E4)
        ctx_t = persist.tile([128, nh // 2, sh], E4)
        mask_sb = persist.tile([128, tt_n], FP)
        eps_sb = persist.tile([128, 1], FP)

        nc.vector.memset(eps_sb, EPS)
        ones_row = persist.tile([1, 64], FP, name="ones_row")
        nc.vector.memset(ones_row, 1.0)
        nc.sync.dma_start(out=mask_sb, in_=mask[:].rearrange("(t p) -> p t", p=128))

        # per-key-tile Schraudolph bias column: mask*A8 + B8
        fe_bcol = persist.tile([128, tt_n], FP, name="fe_bcol")
        nc.vector.tensor_scalar(
            out=fe_bcol,
            in0=mask_sb,
            scalar1=FE8_A,
            scalar2=FE8_B,
            op0=OP.mult,
            op1=OP.add,
        )

        # bias columns for Qt/Kt evictions (partition = output feature in tile)
        bias_cols = {}
        for name in ("bq", "bk"):
            if flags[name]:
                col = persist.tile([128, kt_n], FP, name=f"{name}_col")
                nc.sync.dma_start(
                    out=col, in_=vec_dram[name][:].rearrange("(t p) -> p t", p=128)
                )
                bias_cols[name] = col
        # rows broadcast across partitions for V/out bias and LN affine
        bcast = {}
        for name in ("bv", "bo", "ln_gamma", "ln_beta"):
            if flags[name]:
                t = persist.tile([128, h], FP, name=f"{name}_bc")
                nc.sync.dma_start(out=t, in_=_dram_row_bcast(vec_dram[name], 128, h))
                bcast[name] = t

        # ones columns in V (slot 64 of each 65-wide head block)
        for m in range(tt_n):
            v_view = vsb[:, m, :].rearrange("p (a e) -> p a e", e=65)
            nc.vector.memset(v_view[:, :, 64:65], 1.0)

        with ExitStack() as st_proj:
            xtpool = st_proj.enter_context(tc.tile_pool(name="xtpool", bufs=1))
            wbuf = st_proj.enter_context(tc.tile_pool(name="wbuf", bufs=3))

            xt = xtpool.tile([128, kt_n, s], E4)

            # slot assignment: wv takes slot 0 so the late wo load (issued
            # after the last V fill group) can reuse it; wk/wq live through
            # the whole attention loop (K/Q fill groups read them).
            wv_bf = wbuf.tile([128, kt_n, h], E4, name="wv_bf", tag="w")
            wk_bf = wbuf.tile([128, kt_n, h], E4, name="wk_bf", tag="w")
            wq_bf = wbuf.tile([128, kt_n, h], E4, name="wq_bf", tag="w")

            def load_w(dname, dst):
                for k in range(kt_n):
                    nc.sync.dma_start(
                        out=dst[:, k, :], in_=w_dram[dname][k * 128 : (k + 1) * 128, :]
                    )

            # DMA issue order = priority: xt + wk first (unblocks K tile 0),
            # then wq, then wv. wo is issued much later.
            for k in range(kt_n):
                nc.sync.dma_start(
                    out=xt[:, k, :], in_=xt_d[k * 128 : (k + 1) * 128, :]
                )
            load_w("wk", wk_bf)
            load_w("wq", wq_bf)
            load_w("wv", wv_bf)

            def kq_group(w_bf, dst, bias_col, m, n0, pool, pool_shape, tag, act=False):
                """One K/Q projection PSUM group: 8 accumulating matmuls + evict.

                act=True evicts on the Scalar engine (during attention, DVE is
                busy with exp); upfront evictions stay on DVE.
                """
                n1 = min(n0 + 512, dst.shape[2])
                ps = pool.tile(pool_shape, FP, name="projp", tag=tag)
                for k in range(0, kt_n, 2):
                    nc.tensor.matmul(
                        ps[:, : n1 - n0],
                        w_bf[:, k : k + 2, m * 128 : (m + 1) * 128],
                        xt[:, k : k + 2, n0:n1],
                        start=(k == 0),
                        stop=(k == kt_n - 2),
                        perf_mode=DR,
                    )
                if bias_col is not None:
                    if act:
                        nc.scalar.activation(
                            out=dst[:, m, n0:n1],
                            in_=ps[:, : n1 - n0],
                            func=AF.Identity,
                            bias=bias_col[:, m : m + 1],
                        )
                    else:
                        nc.vector.tensor_scalar_add(
                            out=dst[:, m, n0:n1],
                            in0=ps[:, : n1 - n0],
                            scalar1=bias_col[:, m : m + 1],
                        )
                elif act:
                    nc.scalar.copy(out=dst[:, m, n0:n1], in_=ps[:, : n1 - n0])
                else:
                    nc.vector.tensor_copy(out=dst[:, m, n0:n1], in_=ps[:, : n1 - n0])

            def v_group(m, n0, pool, pool_shape, tag, act=False):
                ps = pool.tile(pool_shape, FP, name="projp", tag=tag)
                for k in range(0, kt_n, 2):
                    nc.tensor.matmul(
                        ps[:, :512],
                        xt[:, k : k + 2, m * 128 : (m + 1) * 128],
                        wv_bf[:, k : k + 2, n0 : n0 + 512],
                        start=(k == 0),
                        stop=(k == kt_n - 2),
                        perf_mode=DR,
                    )
                dst = vsb[:, m, :].rearrange("p (a e) -> p a e", e=65)[
                    :, n0 // 64 : n0 // 64 + 8, 0:64
                ]
                src = ps[:, :512].rearrange("p (a e) -> p a e", e=64)
                if "bv" in bcast:
                    nc.vector.tensor_add(
                        out=dst,
                        in0=src,
                        in1=bcast["bv"][:, n0 : n0 + 512].rearrange(
                            "p (a e) -> p a e", e=64
                        ),
                    )
                elif act:
                    nc.scalar.copy(out=dst, in_=src)
                else:
                    nc.vector.tensor_copy(out=dst, in_=src)

            # ---- upfront projections (overlap the weight DMA) ----
            with tc.tile_pool(name="projps", bufs=2, space="PSUM") as projps:
                for n0 in range(0, s, 512):
                    kq_group(
                        wk_bf, kt, bias_cols.get("bk"), 0, n0, projps, [128, 512], "projp"
                    )
                for n0 in range(0, sh, 512):
                    kq_group(
                        wq_bf, qt, bias_cols.get("bq"), 0, n0, projps, [128, 512], "projp"
                    )
                for m in range(tt_n):
                    v_group(m, 0, projps, [128, 512], "projp")

            # fill tasks, need-by deadline in iteration units (2*tt_n per head
            # pair: tt_n key tiles for each query half)
            n_it_total = (nh // 2) * 2 * tt_n
            it_pair = 2 * tt_n
            tasks = []
            for m in range(1, kt_n):
                for n0 in range(0, s, 512):
                    tasks.append((it_pair * m, "k", m, n0))
                for n0 in range(0, sh, 512):
                    tasks.append((it_pair * m, "q", m, n0))
            for m in range(tt_n):
                # V cols 512.. hold heads 8-15, first consumed by pair nh//4
                tasks.append(((nh // 4) * it_pair + m, "v", m, 512))
            tasks.sort()
            tasks = [
                (min(dl - 16, round((i + 0.5) * n_it_total / len(tasks))), kind, fm, fn0)
                for i, (dl, kind, fm, fn0) in enumerate(tasks)
            ]
            tasks.sort()
            wo_issued = False
            n_v_left = tt_n

            # ---- attention over head pairs, query halves, fills interleaved ----
            # PSUM: 6-slot [128,512] scores-chunk ring (3 key tiles of slack, so
            # gated scores matmuls never leave the PE queue empty) + one
            # [65,512] PV accumulator per head of the pair = exactly 8 banks.
            with ExitStack() as st_att:
                psb = st_att.enter_context(tc.tile_pool(name="psb", bufs=4))
                rpool = st_att.enter_context(tc.tile_pool(name="rpool", bufs=2))
                stps = st_att.enter_context(
                    tc.tile_pool(name="stps", bufs=6, space="PSUM")
                )
                pvps = st_att.enter_context(
                    tc.tile_pool(name="pvps", bufs=1, space="PSUM")
                )
                LOOKAHEAD = 12
                qw = 512  # query-half width

                def run_task(kind, fm, fn0):
                    nonlocal n_v_left, wo_issued
                    if kind == "k":
                        kq_group(
                            wk_bf, kt, bias_cols.get("bk"), fm, fn0, stps, [128, qw],
                            "stp", act=True,
                        )
                    elif kind == "q":
                        kq_group(
                            wq_bf, qt, bias_cols.get("bq"), fm, fn0, stps, [128, qw],
                            "stp", act=True,
                        )
                    else:
                        v_group(fm, fn0, stps, [128, qw], "stp", act=True)
                        n_v_left -= 1
                        if n_v_left == 0 and not wo_issued:
                            wo_issued = True
                            wo_tiles.append(wbuf.tile([128, kt_n, h], E4, name="wo_bf", tag="w"))
                            load_w("wo", wo_tiles[0])

                def exp_tile(stt, dst, m, eng):
                    """PSUM scores [128, qw] -> fp8 exp'd probs, on engine eng."""
                    if eng == "act":
                        nc.scalar.activation(
                            dst,
                            stt,
                            AF.Exp,
                            bias=mask_sb[:, m : m + 1],
                            scale=scale / 256.0,
                        )
                        return
                    nc.vector.tensor_scalar(
                        out=dst.bitcast(mybir.dt.int8),
                        in0=stt,
                        scalar1=FE8_A * scale / 256.0,
                        scalar2=fe_bcol[:, m : m + 1],
                        op0=OP.mult,
                        op1=OP.add,
                    )

                def evict_pv(hh, pv, q0):
                    """Quick-free eviction: one ACT copy per head releases the
                    PV bank; the sum-row DRAM-roundtrip broadcast, reciprocal
                    and normalize run from the SBUF copy, off the PE path."""
                    mt, po = hh // 2, 64 * (hh % 2)
                    pvc = rpool.tile([65, qw], FP, name="pvc", bufs=3)
                    nc.scalar.copy(out=pvc, in_=pv)
                    r_dram = dram.tile([qw], FP, name="rdram", tag="rdram", bufs=3)
                    nc.sync.dma_start(out=r_dram, in_=pvc[64:65, :])
                    # reciprocal cost scales with free size only: fold the qw
                    # sums to [128, qw/128] for the reciprocal, then roundtrip
                    # again to broadcast across 64 partitions.
                    rbt = rpool.tile([128, qw // 128], FP, name="rbt", bufs=3)
                    nc.sync.dma_start(
                        out=rbt, in_=r_dram[:].rearrange("(p t) -> p t", p=128)
                    )
                    rit = rpool.tile([128, qw // 128], FP, name="rit", bufs=3)
                    nc.vector.reciprocal(rit, rbt)
                    r2_dram = dram.tile([qw], FP, name="r2dram", tag="r2dram", bufs=3)
                    nc.sync.dma_start(
                        out=r2_dram[:].rearrange("(p t) -> p t", p=128), in_=rit
                    )
                    rinv = rpool.tile([64, qw], FP, name="rinv", bufs=3)
                    nc.sync.dma_start(
                        out=rinv,
                        in_=bass.AP(
                            tensor=r2_dram.tensor,
                            offset=r2_dram.offset,
                            ap=[[0, 64], [1, qw]],
                        ),
                    )
                    nc.vector.tensor_mul(
                        out=ctx_t[po : po + 64, mt, q0 : q0 + qw],
                        in0=pvc[0:64, :],
                        in1=rinv,
                    )

                def make_pv_group(pvA, pvB, hhA, hhB, p_groups):
                    def pv_group(g, last):
                        """PV accumulation for key tiles (2g, 2g+1): runs LAGGED
                        two iterations behind the exps that produce its p tiles,
                        so the PE never waits on an in-flight exp for it. The
                        final group (g = tt_n/2-1) is carried into the NEXT
                        block's iteration m=1, hiding the block-boundary exp
                        dependency the same way."""
                        pA_, pB_ = p_groups.pop(g)
                        nc.tensor.matmul(
                            pvA,
                            vsb[:, 2 * g : 2 * g + 2, hhA * 65 : (hhA + 1) * 65],
                            pA_[:, 0:2, :],
                            start=(g == 0),
                            stop=last,
                            perf_mode=DR,
                        )
                        nc.tensor.matmul(
                            pvB,
                            vsb[:, 2 * g : 2 * g + 2, hhB * 65 : (hhB + 1) * 65],
                            pB_[:, 0:2, :],
                            start=(g == 0),
                            stop=last,
                            perf_mode=DR,
                        )

                    return pv_group

                def evict_pv_fast(hh, pv, q0):
                    """Low-latency eviction for the final block: reciprocal of
                    the sum row + PE broadcast across 64 partitions replaces
                    the 4-hop DMA roundtrip (~2us vs ~10us), so the output
                    projection's last chains aren't left waiting."""
                    mt, po = hh // 2, 64 * (hh % 2)
                    pvc = rpool.tile([65, qw], FP, name="pvc", bufs=3)
                    nc.scalar.copy(out=pvc, in_=pv)
                    rrow = rpool.tile([1, qw], FP, name="rrow", bufs=2)
                    nc.vector.reciprocal(rrow, pvc[64:65, :])
                    rps = stps.tile([128, qw], FP, name="stp", tag="stp")
                    nc.tensor.matmul(
                        rps[0:64, :], ones_row, rrow, start=True, stop=True
                    )
                    nc.vector.tensor_mul(
                        out=ctx_t[po : po + 64, mt, q0 : q0 + qw],
                        in0=pvc[0:64, :],
                        in1=rps[0:64, :],
                    )

                wo_tiles = []
                it = 0
                blocks = [
                    (2 * j, 2 * j + 1, q0)
                    for j in range(nh // 2)
                    for q0 in range(0, sh, qw)
                ]
                for bi, (hhA, hhB, q0) in enumerate(blocks):
                    j = hhA // 2
                    p_groups = {}
                    pvA = pvps.tile([65, qw], FP, name="pvA")
                    pvB = pvps.tile([65, qw], FP, name="pvB")
                    pv_group = make_pv_group(pvA, pvB, hhA, hhB, p_groups)
                    for m in range(tt_n):
                        # ungated PE work first, so the engine queue never
                        # drains while a gated scores matmul waits on its slot
                        # (a drained queue re-throttles HAM, and the next
                        # matmuls run cold at half rate).
                        while tasks and tasks[0][0] <= it:
                            _, kind, fm, fn0 = tasks.pop(0)
                            run_task(kind, fm, fn0)
                        if tasks and tasks[0][0] <= it + LOOKAHEAD:
                            _, kind, fm, fn0 = tasks.pop(0)
                            run_task(kind, fm, fn0)
                        if m % 2 == 1 and m >= 3:
                            pv_group((m - 3) // 2, last=False)
                        # paired scores: head A on PE rows 0-63, head B on rows
                        # 64-127 -> with both matmuls adjacent and in distinct
                        # PSUM banks the PE runs them concurrently.
                        sttA = stps.tile([128, qw], FP, name="stp", tag="stp")
                        sttB = stps.tile([128, qw], FP, name="stp", tag="stp")
                        nc.tensor.matmul(
                            sttA,
                            kt[0:64, j, m * 128 : (m + 1) * 128],
                            qt[0:64, j, q0 : q0 + qw],
                            start=True,
                            stop=True,
                        )
                        nc.tensor.matmul(
                            sttB,
                            kt[64:128, j, m * 128 : (m + 1) * 128],
                            qt[64:128, j, q0 : q0 + qw],
                            start=True,
                            stop=True,
                        )
                        if m % 2 == 0:
                            pA = psb.tile([128, 2, qw], E4, name="pexpA")
                            pB = psb.tile([128, 2, qw], E4, name="pexpB")
                            p_groups[m // 2] = (pA, pB)
                        # one head per engine every key tile, so the two exp
                        # walls run in parallel; alternate which head gets the
                        # exact ACT exp so the Schraudolph error spreads evenly.
                        engs = ("act", "dve") if m % 2 == 0 else ("dve", "act")
                        exp_tile(sttA, pA[:, m % 2, :], m, engs[0])
                        exp_tile(sttB, pB[:, m % 2, :], m, engs[1])
                        it += 1
                    pv_group(tt_n // 2 - 1, last=True)
                    ev = evict_pv_fast if bi == len(blocks) - 1 else evict_pv
                    ev(hhA, pvA, q0)
                    ev(hhB, pvB, q0)
                for _, kind, fm, fn0 in tasks:  # leftovers (shouldn't happen)
                    run_task(kind, fm, fn0)
            wo_bf = wo_tiles[0]

            # ---- output projection + residual + LayerNorm (natural layout) ----
            with (
                tc.tile_pool(name="ops", bufs=8, space="PSUM") as ops,
                tc.tile_pool(name="xrp", bufs=qt_n) as xrp,
                tc.tile_pool(name="osb", bufs=3) as osb,
                tc.tile_pool(name="lnp", bufs=4) as lnp,
            ):
                xres_tiles = []
                for m in range(qt_n):
                    xr = xrp.tile([128, h], FP, name="xres", tag="xres")
                    nc.sync.dma_start(out=xr, in_=x[m * 128 : (m + 1) * 128, :])
                    xres_tiles.append(xr)
                # chains for the first 4 row-tiles run their early head-pairs
                # ahead of time: the final pair needs the last heads' ctx
                # (gated on the softmax-sum roundtrip), and running the
                # independent pairs first overlaps that latency.
                early = {}
                for m in range(0, min(4, qt_n)):
                    for n0 in range(0, h, 512):
                        ps = ops.tile([128, 512], FP, name="op")
                        for mt in range(0, nh // 2 - 2, 2):
                            nc.tensor.matmul(
                                ps,
                                ctx_t[:, mt : mt + 2, m * 128 : (m + 1) * 128],
                                wo_bf[:, mt : mt + 2, n0 : n0 + 512],
                                start=(mt == 0),
                                stop=False,
                                perf_mode=DR,
                            )
                        early[(m, n0)] = ps
                for m in range(qt_n):
                    pss = []
                    for n0 in range(0, h, 512):
                        if (m, n0) in early:
                            ps = early[(m, n0)]
                            mt = nh // 2 - 2
                            nc.tensor.matmul(
                                ps,
                                ctx_t[:, mt : mt + 2, m * 128 : (m + 1) * 128],
                                wo_bf[:, mt : mt + 2, n0 : n0 + 512],
                                start=False,
                                stop=True,
                                perf_mode=DR,
                            )
                        else:
                            ps = ops.tile([128, 512], FP, name="op")
                            # ctx_t tile mt holds heads 2mt / 2mt+1 on partitions
                            # 0-63 / 64-127, matching Wo rows mt*128..(mt+1)*128.
                            for mt in range(0, nh // 2, 2):
                                nc.tensor.matmul(
                                    ps,
                                    ctx_t[:, mt : mt + 2, m * 128 : (m + 1) * 128],
                                    wo_bf[:, mt : mt + 2, n0 : n0 + 512],
                                    start=(mt == 0),
                                    stop=(mt == nh // 2 - 2),
                                    perf_mode=DR,
                                )
                        pss.append((n0, ps))
                    xres = xres_tiles[m]
                    o = osb.tile([128, h], FP, name="osum")
                    for n0, ps in pss:
                        nc.vector.tensor_add(
                            out=o[:, n0 : n0 + 512], in0=ps, in1=xres[:, n0 : n0 + 512]
                        )
                    if "bo" in bcast:
                        nc.vector.tensor_add(out=o, in0=o, in1=bcast["bo"])
                    nsub = (h + 511) // 512
                    stats = lnp.tile([128, nsub, 6], FP, name="stats")
                    for i in range(nsub):
                        nc.vector.bn_stats(
                            out=stats[:, i, :], in_=o[:, i * 512 : (i + 1) * 512]
                        )
                    mv = lnp.tile([128, 2], FP, name="mv")
                    nc.vector.bn_aggr(out=mv, in_=stats)
                    std = lnp.tile([128, 1], FP, name="std")
                    nc.scalar.activation(std, mv[:, 1:2], AF.Sqrt, bias=eps_sb)
                    inv = lnp.tile([128, 1], FP, name="inv")
                    nc.vector.reciprocal(inv, std)
                    nb = lnp.tile([128, 1], FP, name="nb")
                    nc.vector.tensor_tensor(
                        out=nb, in0=mv[:, 0:1], in1=inv, op=OP.mult
                    )
                    nc.vector.tensor_scalar_mul(out=nb, in0=nb, scalar1=-1.0)
                    y = osb.tile([128, h], FP, name="yout")
                    nc.scalar.activation(
                        out=y, in_=o, func=AF.Identity, bias=nb, scale=inv
                    )
                    if "ln_gamma" in bcast:
                        nc.vector.tensor_mul(out=y, in0=y, in1=bcast["ln_gamma"])
                    if "ln_beta" in bcast:
                        nc.vector.tensor_add(out=y, in0=y, in1=bcast["ln_beta"])
                    nc.sync.dma_start(out=out[m * 128 : (m + 1) * 128, :], in_=y)

    _split_sync_waits(nc)
    return nc


_NC_CACHE = {}


def _get_nc(s, h, nh, sh, flags):
    key = (s, h, nh, sh, tuple(sorted(flags.items())))
    if key not in _NC_CACHE:
        _NC_CACHE[key] = _build(s, h, nh, sh, flags)
    return _NC_CACHE[key]


def _prepare(hidden_states, attention_mask, Wq, bq, Wk, bk, Wv, bv, Wo, bo, ln_gamma, ln_beta):
    hs = np.ascontiguousarray(np.asarray(hidden_states, dtype=np.float32))
    b_, s_, h_ = hs.shape
    nh_ = h_ // 64
    sh_ = s_ // 2
    am = np.asarray(attention_mask, dtype=np.float32).reshape(b_, s_)
    flags = {
        "bq": bool(np.any(np.asarray(bq))),
        "bk": bool(np.any(np.asarray(bk))),
        "bv": bool(np.any(np.asarray(bv))),
        "bo": bool(np.any(np.asarray(bo))),
        "ln_gamma": not bool(np.all(np.asarray(ln_gamma) == 1.0)),
        "ln_beta": bool(np.any(np.asarray(ln_beta))),
    }
    nc = _get_nc(s_, h_, nh_, sh_, flags)

    f32c = lambda a: np.ascontiguousarray(np.asarray(a, dtype=np.float32))
    f8c = lambda a, sc: np.ascontiguousarray(
        (np.asarray(a, dtype=np.float32) * sc).astype(ml_dtypes.float8_e4m3fn)
    )
    # weights x16 in fp8 (keeps small values out of the subnormal range);
    # K/Q both carry x16 so scores carry x256, folded into the Exp scale.
    # ctx_t carries x64 (x16 from V, x4 from the sum eviction), Wo x16, so
    # the out-proj PSUM carries x1024 — matched by scaling the residual
    # x1024 on the host. LayerNorm is scale-invariant, so the output is
    # unchanged.
    shared = {
        "wq": f8c(Wq, 16.0),
        "wk": f8c(Wk, 16.0),
        "wv": f8c(Wv, 64.0),
        "wo": f8c(Wo, 16.0),
    }
    scales = {"bq": 16.0, "bk": 16.0, "bv": 64.0, "bo": 1024.0}
    for name, arr in (
        ("bq", bq),
        ("bk", bk),
        ("bv", bv),
        ("bo", bo),
        ("ln_gamma", ln_gamma),
        ("ln_beta", ln_beta),
    ):
        if flags[name]:
            shared[name] = f32c(np.asarray(arr) * scales.get(name, 1.0))

    in_maps = []
    for c in range(N_CORES):
        bb, half = c // 2, c % 2
        mine = slice(half * sh_, (half + 1) * sh_)
        other = slice((1 - half) * sh_, (2 - half) * sh_)
        xp = np.concatenate([hs[bb, mine], hs[bb, other]], axis=0)
        xt = np.ascontiguousarray(xp.T.astype(ml_dtypes.float8_e4m3fn))
        mp = np.ascontiguousarray(np.concatenate([am[bb, mine], am[bb, other]]))
        in_maps.append(
            {
                "x": np.ascontiguousarray(xp[:sh_] * 1024.0),
                "xt": xt,
                "mask": mp,
                **shared,
            }
        )
    return nc, in_maps, (b_, s_, h_, sh_)


def _assemble(results, shape):
    b_, s_, h_, sh_ = shape
    out = np.empty((b_, s_, h_), dtype=np.float32)
    for c in range(N_CORES):
        bb, half = c // 2, c % 2
        out[bb, half * sh_ : (half + 1) * sh_] = results[c]["out"]
    return out


def kernel(**inputs) -> np.ndarray:
    nc, in_maps, shape = _prepare(**inputs)
    res = run_bass_kernel_spmd(nc, in_maps, core_ids=list(range(N_CORES)))
    return _assemble(res.results, shape)


# revision 25
# speedup vs baseline: 1.0140x; 1.0140x over previous
"""BertAttention (QKV proj + MHA + output proj + residual + LayerNorm) on 8 TRN2 NeuronCores.

Sharding: batch (4-way) x query-sequence-half (2-way) => 8 shards, no collectives.
Core c handles batch b=c//2, query half c%2. Each core computes K/V for its full
batch sequence (all heads) and Q/attention/output-proj/LayerNorm for its 1024
query rows. K/V projection work is duplicated across the 2 cores sharing a batch;
in exchange there is zero cross-core communication.

The host permutes each core's X rows so its query half comes first — attention is
permutation-invariant over keys as long as (K, V, mask) share the permutation, so
the program is identical across cores (pure SPMD) with no per-core indices.

Host pre-stages inputs: X transposed to [H, S] fp8 (feature on partitions after
DMA), weights cast to fp8, residual rows kept fp32. This removes all on-device
casts and PE transposes and cuts the load DMA bytes.

Layouts (SBUF partition dim first):
  xt:      [128, H/128, S]   transposed activations, fp8 (direct DMA)
  Kt:      [128, H/128, S]   transposed keys (feature on partitions), bf16
  Qt:      [128, H/128, SH]  transposed, bf16
  V:       [128, S/128, NH*65] natural ([tok, head-dim]) with a ones column per
           head at slot 64 — the PV matmul then yields sum(exp) as row 64 for free
  scores:  St[ktok, qtok] in PSUM; softmax sum over ktok (the partition dim) comes
           from the ones-column trick; max-subtraction safely skipped (|s| <~ 1)
  ctx:     [128, NH/2, SH]   transposed (head dim on partitions), fp8
  out:     natural [qtok, H] — residual add + LayerNorm along the free dim.

Attention runs over HEAD PAIRS (2j, 2j+1) x QUERY HALVES: head 2j lives on
partitions 0-63 of its Kt/Qt tile, head 2j+1 on partitions 64-127, so the two
scores matmuls are 64-contraction tiles at PE row positions 0 and 64 — the PE
runs them CONCURRENTLY (tile concurrency, measured 1.7x) when the engine
stays dense. Query-halving shrinks the working set so PSUM fits a SIX-slot
[128,512] scores-chunk rotation (6 banks, 3 key tiles of slack) plus one
[65,512] PV accumulator per head of the pair (2 banks). The deep rotation is
what keeps the PE queue from draining while exps catch up — a drained queue
re-throttles HAM and every matmul then runs ~2x slower (the failure mode of
shallower layouts).

The softmax exp (the largest non-PE cost) is split one head per engine every
key tile: ACT (native Exp) and DVE (e4m3-domain Schraudolph: ONE tensor_scalar
computing int8(y*8/ln2 + 55.63) through a bitcast view of the fp8 p tile —
valid fp8e4m3 bits, max 7.3% elem error, cancelled by softmax normalization).
PV matmuls run LAGGED two iterations behind the exps that produce their p
tiles (p tiles persist in SBUF rings), so they never gate on an in-flight exp.
Projection work (K/Q/V + late Wo) is deadline-paced into the loop as PE
gap-filler, borrowing scores-rotation slots; its PSUM evictions run on ACT
during attention (DVE is exp-saturated).
"""

from contextlib import ExitStack

import numpy as np
import ml_dtypes

import bass_rust
import concourse.bass as bass
import concourse.mybir as mybir
from concourse.tile import TileContext
from concourse.bass_utils import run_bass_kernel_spmd

FP = mybir.dt.float32
BF = mybir.dt.bfloat16
E4 = mybir.dt.float8e4
DR = mybir.MatmulPerfMode.DoubleRow
AF = mybir.ActivationFunctionType
OP = mybir.AluOpType

N_CORES = 8
EPS = 1e-12

# e4m3-domain Schraudolph fast-exp: e4m3_bits(exp(y)) ~ int8(y*8/ln2 + B8),
# one DVE tensor_scalar (float->int8 convert) writing through a bitcast view
# of the fp8 p tile. Host-swept constant: max elem rel err 7.3% (the fp32
# Schraudolph + fp8 cast it replaces measured 8.0%), cancelled by softmax
# normalization. (GpSimd measured ~4.3us per [128,1024] converting cast —
# not viable for bulk elementwise work, so exp runs on ACT and DVE only.)
FE8_A = 8.0 / float(np.log(2.0))
FE8_B = 55.63

# The walrus build in this toolchain rejects instructions that carry more than
# one sync-wait command ("Too many sync wait commands", CoreV2/V3 setupSyncWait),
# while Tile freely attaches several semaphore waits to one instruction (and the
# TileContext exit drain aggregates one wait per logical processor). Hoist the
# excess waits onto standalone InstEventSemaphore carriers on the same engine,
# placed immediately before the instruction — engine streams are serial, so the
# gating semantics are identical.
_MAX_WAITS_PER_INST = 1


def _split_sync_waits(nc, cap=_MAX_WAITS_PER_INST):
    n_split = 0
    for fn in nc.m.functions:
        for bb in fn.blocks:
            insts = list(bb.instructions)
            out = []
            changed = False
            for ins in insts:
                si = ins.sync_info
                waits = list(si.on_wait) if (si is not None and si.on_wait) else []
                if len(waits) > cap:
                    head, tail = waits[: len(waits) - cap], waits[len(waits) - cap :]
                    for j, w in enumerate(head):
                        ev = mybir.InstEventSemaphore(
                            name=f"{ins.name}-sw{j}",
                            engine=ins.engine,
                            ins=[],
                            outs=[],
                            sync_info=bass_rust.SyncInfo(on_wait=[w], on_update=[]),
                        )
                        out.append(ev)
                        n_split += 1
                    si.on_wait = tail
                    changed = True
                out.append(ins)
            if changed:
                bb.instructions[:] = out
    return n_split


def _dram_row_bcast(handle, p, n):
    """AP reading DRAM vector [n] broadcast across p partitions."""
    return bass.AP(tensor=handle, offset=0, ap=[[0, p], [1, n]])


def _build(s, h, nh, sh, flags):
    """Build the per-core Bass program. flags: which bias/affine inputs matter."""
    hd = h // nh
    assert hd == 64, "head packing assumes head_dim 64 (2 heads per 128 partitions)"
    kt_n = h // 128  # contraction tiles over hidden dim
    tt_n = s // 128  # key-token tiles
    qt_n = sh // 128  # query-token tiles
    scale = 1.0 / float(np.sqrt(hd))

    nc = bass.Bass(target_bir_lowering=False)
    x = nc.dram_tensor("x", [sh, h], FP, kind="ExternalInput")  # residual rows
    xt_d = nc.dram_tensor("xt", [h, s], E4, kind="ExternalInput")  # transposed
    mask = nc.dram_tensor("mask", [s], FP, kind="ExternalInput")
    w_dram = {
        n: nc.dram_tensor(n, [h, h], E4, kind="ExternalInput")
        for n in ("wq", "wk", "wv", "wo")
    }
    vec_dram = {
        n: nc.dram_tensor(n, [h], FP, kind="ExternalInput")
        for n in ("bq", "bk", "bv", "bo", "ln_gamma", "ln_beta")
        if flags[n]
    }
    out = nc.dram_tensor("out", [sh, h], FP, kind="ExternalOutput")

    with TileContext(nc) as tc, ExitStack() as st_all:
        persist = st_all.enter_context(tc.tile_pool(name="persist", bufs=1))
        dram = st_all.enter_context(tc.tile_pool(name="dram", bufs=1, space="DRAM"))
        qt = persist.tile([128, kt_n, sh], BF)
        kt = persist.tile([128, kt_n, s], BF)
        vsb = persist.tile([128, tt_n, nh * 65], E4)
        ctx_t = persist.tile([128, nh // 2, sh], E4)
        mask_sb = persist.tile([128, tt_n], FP)
        eps_sb = persist.tile([128, 1], FP)

        nc.vector.memset(eps_sb, EPS)
        nc.sync.dma_start(out=mask_sb, in_=mask[:].rearrange("(t p) -> p t", p=128))

        # per-key-tile Schraudolph bias column: mask*A8 + B8
        fe_bcol = persist.tile([128, tt_n], FP, name="fe_bcol")
        nc.vector.tensor_scalar(
            out=fe_bcol,
            in0=mask_sb,
            scalar1=FE8_A,
            scalar2=FE8_B,
            op0=OP.mult,
            op1=OP.add,
        )

        # bias columns for Qt/Kt evictions (partition = output feature in tile)
        bias_cols = {}
        for name in ("bq", "bk"):
            if flags[name]:
                col = persist.tile([128, kt_n], FP, name=f"{name}_col")
                nc.sync.dma_start(
                    out=col, in_=vec_dram[name][:].rearrange("(t p) -> p t", p=128)
                )
                bias_cols[name] = col
        # rows broadcast across partitions for V/out bias and LN affine
        bcast = {}
        for name in ("bv", "bo", "ln_gamma", "ln_beta"):
            if flags[name]:
                t = persist.tile([128, h], FP, name=f"{name}_bc")
                nc.sync.dma_start(out=t, in_=_dram_row_bcast(vec_dram[name], 128, h))
                bcast[name] = t

        # ones columns in V (slot 64 of each 65-wide head block)
        for m in range(tt_n):
            v_view = vsb[:, m, :].rearrange("p (a e) -> p a e", e=65)
            nc.vector.memset(v_view[:, :, 64:65], 1.0)

        with ExitStack() as st_proj:
            xtpool = st_proj.enter_context(tc.tile_pool(name="xtpool", bufs=1))
            wbuf = st_proj.enter_context(tc.tile_pool(name="wbuf", bufs=3))

            xt = xtpool.tile([128, kt_n, s], E4)

            # slot assignment: wv takes slot 0 so the late wo load (issued
            # after the last V fill group) can reuse it; wk/wq live through
            # the whole attention loop (K/Q fill groups read them).
            wv_bf = wbuf.tile([128, kt_n, h], E4, name="wv_bf", tag="w")
            wk_bf = wbuf.tile([128, kt_n, h], E4, name="wk_bf", tag="w")
            wq_bf = wbuf.tile([128, kt_n, h], E4, name="wq_bf", tag="w")

            def load_w(dname, dst):
                for k in range(kt_n):
                    nc.sync.dma_start(
                        out=dst[:, k, :], in_=w_dram[dname][k * 128 : (k + 1) * 128, :]
                    )

            # DMA issue order = priority: xt + wk first (unblocks K tile 0),
            # then wq, then wv. wo is issued much later.
            for k in range(kt_n):
                nc.sync.dma_start(
                    out=xt[:, k, :], in_=xt_d[k * 128 : (k + 1) * 128, :]
                )
            load_w("wk", wk_bf)
            load_w("wq", wq_bf)
            load_w("wv", wv_bf)

            def kq_group(w_bf, dst, bias_col, m, n0, pool, pool_shape, tag, act=False):
                """One K/Q projection PSUM group: 8 accumulating matmuls + evict.

                act=True evicts on the Scalar engine (during attention, DVE is
                busy with exp); upfront evictions stay on DVE.
                """
                n1 = min(n0 + 512, dst.shape[2])
                ps = pool.tile(pool_shape, FP, name="projp", tag=tag)
                for k in range(0, kt_n, 2):
                    nc.tensor.matmul(
                        ps[:, : n1 - n0],
                        w_bf[:, k : k + 2, m * 128 : (m + 1) * 128],
                        xt[:, k : k + 2, n0:n1],
                        start=(k == 0),
                        stop=(k == kt_n - 2),
                        perf_mode=DR,
                    )
                if bias_col is not None:
                    if act:
                        nc.scalar.activation(
                            out=dst[:, m, n0:n1],
                            in_=ps[:, : n1 - n0],
                            func=AF.Identity,
                            bias=bias_col[:, m : m + 1],
                        )
                    else:
                        nc.vector.tensor_scalar_add(
                            out=dst[:, m, n0:n1],
                            in0=ps[:, : n1 - n0],
                            scalar1=bias_col[:, m : m + 1],
                        )
                elif act:
                    nc.scalar.copy(out=dst[:, m, n0:n1], in_=ps[:, : n1 - n0])
                else:
                    nc.vector.tensor_copy(out=dst[:, m, n0:n1], in_=ps[:, : n1 - n0])

            def v_group(m, n0, pool, pool_shape, tag, act=False):
                ps = pool.tile(pool_shape, FP, name="projp", tag=tag)
                for k in range(0, kt_n, 2):
                    nc.tensor.matmul(
                        ps[:, :512],
                        xt[:, k : k + 2, m * 128 : (m + 1) * 128],
                        wv_bf[:, k : k + 2, n0 : n0 + 512],
                        start=(k == 0),
                        stop=(k == kt_n - 2),
                        perf_mode=DR,
                    )
                dst = vsb[:, m, :].rearrange("p (a e) -> p a e", e=65)[
                    :, n0 // 64 : n0 // 64 + 8, 0:64
                ]
                src = ps[:, :512].rearrange("p (a e) -> p a e", e=64)
                if "bv" in bcast:
                    nc.vector.tensor_add(
                        out=dst,
                        in0=src,
                        in1=bcast["bv"][:, n0 : n0 + 512].rearrange(
                            "p (a e) -> p a e", e=64
                        ),
                    )
                elif act:
                    nc.scalar.copy(out=dst, in_=src)
                else:
                    nc.vector.tensor_copy(out=dst, in_=src)

            # ---- upfront projections (overlap the weight DMA) ----
            with tc.tile_pool(name="projps", bufs=2, space="PSUM") as projps:
                for n0 in range(0, s, 512):
                    kq_group(
                        wk_bf, kt, bias_cols.get("bk"), 0, n0, projps, [128, 512], "projp"
                    )
                for n0 in range(0, sh, 512):
                    kq_group(
                        wq_bf, qt, bias_cols.get("bq"), 0, n0, projps, [128, 512], "projp"
                    )
                for m in range(tt_n):
                    v_group(m, 0, projps, [128, 512], "projp")

            # fill tasks, need-by deadline in iteration units (2*tt_n per head
            # pair: tt_n key tiles for each query half)
            n_it_total = (nh // 2) * 2 * tt_n
            it_pair = 2 * tt_n
            tasks = []
            for m in range(1, kt_n):
                for n0 in range(0, s, 512):
                    tasks.append((it_pair * m, "k", m, n0))
                for n0 in range(0, sh, 512):
                    tasks.append((it_pair * m, "q", m, n0))
            for m in range(tt_n):
                # V cols 512.. hold heads 8-15, first consumed by pair nh//4
                tasks.append(((nh // 4) * it_pair + m, "v", m, 512))
            tasks.sort()
            tasks = [
                (min(dl - 16, round((i + 0.5) * n_it_total / len(tasks))), kind, fm, fn0)
                for i, (dl, kind, fm, fn0) in enumerate(tasks)
            ]
            tasks.sort()
            wo_issued = False
            n_v_left = tt_n

            # ---- attention over head pairs, query halves, fills interleaved ----
            # PSUM: 6-slot [128,512] scores-chunk ring (3 key tiles of slack, so
            # gated scores matmuls never leave the PE queue empty) + one
            # [65,512] PV accumulator per head of the pair = exactly 8 banks.
            with ExitStack() as st_att:
                psb = st_att.enter_context(tc.tile_pool(name="psb", bufs=4))
                rpool = st_att.enter_context(tc.tile_pool(name="rpool", bufs=2))
                stps = st_att.enter_context(
                    tc.tile_pool(name="stps", bufs=6, space="PSUM")
                )
                pvps = st_att.enter_context(
                    tc.tile_pool(name="pvps", bufs=1, space="PSUM")
                )
                LOOKAHEAD = 12
                qw = 512  # query-half width

                def run_task(kind, fm, fn0):
                    nonlocal n_v_left, wo_issued
                    if kind == "k":
                        kq_group(
                            wk_bf, kt, bias_cols.get("bk"), fm, fn0, stps, [128, qw],
                            "stp", act=True,
                        )
                    elif kind == "q":
                        kq_group(
                            wq_bf, qt, bias_cols.get("bq"), fm, fn0, stps, [128, qw],
                            "stp", act=True,
                        )
                    else:
                        v_group(fm, fn0, stps, [128, qw], "stp", act=True)
                        n_v_left -= 1
                        if n_v_left == 0 and not wo_issued:
                            wo_issued = True
                            wo_tiles.append(wbuf.tile([128, kt_n, h], E4, name="wo_bf", tag="w"))
                            load_w("wo", wo_tiles[0])

                def exp_tile(stt, dst, m, eng):
                    """PSUM scores [128, qw] -> fp8 exp'd probs, on engine eng."""
                    if eng == "act":
                        nc.scalar.activation(
                            dst,
                            stt,
                            AF.Exp,
                            bias=mask_sb[:, m : m + 1],
                            scale=scale / 256.0,
                        )
                        return
                    nc.vector.tensor_scalar(
                        out=dst.bitcast(mybir.dt.int8),
                        in0=stt,
                        scalar1=FE8_A * scale / 256.0,
                        scalar2=fe_bcol[:, m : m + 1],
                        op0=OP.mult,
                        op1=OP.add,
                    )

                def evict_pv(hh, pv, q0):
                    """Quick-free eviction: one ACT copy per head releases the
                    PV bank; the sum-row DRAM-roundtrip broadcast, reciprocal
                    and normalize run from the SBUF copy, off the PE path."""
                    mt, po = hh // 2, 64 * (hh % 2)
                    pvc = rpool.tile([65, qw], FP, name="pvc", bufs=3)
                    nc.scalar.copy(out=pvc, in_=pv)
                    r_dram = dram.tile([qw], FP, name="rdram", tag="rdram", bufs=3)
                    nc.sync.dma_start(out=r_dram, in_=pvc[64:65, :])
                    # reciprocal cost scales with free size only: fold the qw
                    # sums to [128, qw/128] for the reciprocal, then roundtrip
                    # again to broadcast across 64 partitions.
                    rbt = rpool.tile([128, qw // 128], FP, name="rbt", bufs=3)
                    nc.sync.dma_start(
                        out=rbt, in_=r_dram[:].rearrange("(p t) -> p t", p=128)
                    )
                    rit = rpool.tile([128, qw // 128], FP, name="rit", bufs=3)
                    nc.vector.reciprocal(rit, rbt)
                    r2_dram = dram.tile([qw], FP, name="r2dram", tag="r2dram", bufs=3)
                    nc.sync.dma_start(
                        out=r2_dram[:].rearrange("(p t) -> p t", p=128), in_=rit
                    )
                    rinv = rpool.tile([64, qw], FP, name="rinv", bufs=3)
                    nc.sync.dma_start(
                        out=rinv,
                        in_=bass.AP(
                            tensor=r2_dram.tensor,
                            offset=r2_dram.offset,
                            ap=[[0, 64], [1, qw]],
                        ),
                    )
                    nc.vector.tensor_mul(
                        out=ctx_t[po : po + 64, mt, q0 : q0 + qw],
                        in0=pvc[0:64, :],
                        in1=rinv,
                    )

                def make_pv_group(pvA, pvB, hhA, hhB, p_groups):
                    def pv_group(g, last):
                        """PV accumulation for key tiles (2g, 2g+1): runs LAGGED
                        two iterations behind the exps that produce its p tiles,
                        so the PE never waits on an in-flight exp for it. The
                        final group (g = tt_n/2-1) is carried into the NEXT
                        block's iteration m=1, hiding the block-boundary exp
                        dependency the same way."""
                        pA_, pB_ = p_groups.pop(g)
                        nc.tensor.matmul(
                            pvA,
                            vsb[:, 2 * g : 2 * g + 2, hhA * 65 : (hhA + 1) * 65],
                            pA_[:, 0:2, :],
                            start=(g == 0),
                            stop=last,
                            perf_mode=DR,
                        )
                        nc.tensor.matmul(
                            pvB,
                            vsb[:, 2 * g : 2 * g + 2, hhB * 65 : (hhB + 1) * 65],
                            pB_[:, 0:2, :],
                            start=(g == 0),
                            stop=last,
                            perf_mode=DR,
                        )

                    return pv_group

                wo_tiles = []
                it = 0
                blocks = [
                    (2 * j, 2 * j + 1, q0)
                    for j in range(nh // 2)
                    for q0 in range(0, sh, qw)
                ]
                for bi, (hhA, hhB, q0) in enumerate(blocks):
                    j = hhA // 2
                    p_groups = {}
                    pvA = pvps.tile([65, qw], FP, name="pvA")
                    pvB = pvps.tile([65, qw], FP, name="pvB")
                    pv_group = make_pv_group(pvA, pvB, hhA, hhB, p_groups)
                    for m in range(tt_n):
                        # ungated PE work first, so the engine queue never
                        # drains while a gated scores matmul waits on its slot
                        # (a drained queue re-throttles HAM, and the next
                        # matmuls run cold at half rate).
                        while tasks and tasks[0][0] <= it:
                            _, kind, fm, fn0 = tasks.pop(0)
                            run_task(kind, fm, fn0)
                        if tasks and tasks[0][0] <= it + LOOKAHEAD:
                            _, kind, fm, fn0 = tasks.pop(0)
                            run_task(kind, fm, fn0)
                        if m % 2 == 1 and m >= 3:
                            pv_group((m - 3) // 2, last=False)
                        # paired scores: head A on PE rows 0-63, head B on rows
                        # 64-127 -> with both matmuls adjacent and in distinct
                        # PSUM banks the PE runs them concurrently.
                        sttA = stps.tile([128, qw], FP, name="stp", tag="stp")
                        sttB = stps.tile([128, qw], FP, name="stp", tag="stp")
                        nc.tensor.matmul(
                            sttA,
                            kt[0:64, j, m * 128 : (m + 1) * 128],
                            qt[0:64, j, q0 : q0 + qw],
                            start=True,
                            stop=True,
                        )
                        nc.tensor.matmul(
                            sttB,
                            kt[64:128, j, m * 128 : (m + 1) * 128],
                            qt[64:128, j, q0 : q0 + qw],
                            start=True,
                            stop=True,
                        )
                        if m % 2 == 0:
                            pA = psb.tile([128, 2, qw], E4, name="pexpA")
                            pB = psb.tile([128, 2, qw], E4, name="pexpB")
                            p_groups[m // 2] = (pA, pB)
                        # one head per engine every key tile, so the two exp
                        # walls run in parallel; alternate which head gets the
                        # exact ACT exp so the Schraudolph error spreads evenly.
                        engs = ("act", "dve") if m % 2 == 0 else ("dve", "act")
                        exp_tile(sttA, pA[:, m % 2, :], m, engs[0])
                        exp_tile(sttB, pB[:, m % 2, :], m, engs[1])
                        it += 1
                    pv_group(tt_n // 2 - 1, last=True)
                    evict_pv(hhA, pvA, q0)
                    evict_pv(hhB, pvB, q0)
                for _, kind, fm, fn0 in tasks:  # leftovers (shouldn't happen)
                    run_task(kind, fm, fn0)
            wo_bf = wo_tiles[0]

            # ---- output projection + residual + LayerNorm (natural layout) ----
            with (
                tc.tile_pool(name="ops", bufs=8, space="PSUM") as ops,
                tc.tile_pool(name="xrp", bufs=qt_n) as xrp,
                tc.tile_pool(name="osb", bufs=3) as osb,
                tc.tile_pool(name="lnp", bufs=4) as lnp,
            ):
                xres_tiles = []
                for m in range(qt_n):
                    xr = xrp.tile([128, h], FP, name="xres", tag="xres")
                    nc.sync.dma_start(out=xr, in_=x[m * 128 : (m + 1) * 128, :])
                    xres_tiles.append(xr)
                # chains for the first 4 row-tiles run their early head-pairs
                # ahead of time: the final pair needs the last heads' ctx
                # (gated on the softmax-sum roundtrip), and running the
                # independent pairs first overlaps that latency.
                early = {}
                for m in range(0, min(4, qt_n)):
                    for n0 in range(0, h, 512):
                        ps = ops.tile([128, 512], FP, name="op")
                        for mt in range(0, nh // 2 - 2, 2):
                            nc.tensor.matmul(
                                ps,
                                ctx_t[:, mt : mt + 2, m * 128 : (m + 1) * 128],
                                wo_bf[:, mt : mt + 2, n0 : n0 + 512],
                                start=(mt == 0),
                                stop=False,
                                perf_mode=DR,
                            )
                        early[(m, n0)] = ps
                for m in range(qt_n):
                    pss = []
                    for n0 in range(0, h, 512):
                        if (m, n0) in early:
                            ps = early[(m, n0)]
                            mt = nh // 2 - 2
                            nc.tensor.matmul(
                                ps,
                                ctx_t[:, mt : mt + 2, m * 128 : (m + 1) * 128],
                                wo_bf[:, mt : mt + 2, n0 : n0 + 512],
                                start=False,
                                stop=True,
                                perf_mode=DR,
                            )
                        else:
                            ps = ops.tile([128, 512], FP, name="op")
                            # ctx_t tile mt holds heads 2mt / 2mt+1 on partitions
                            # 0-63 / 64-127, matching Wo rows mt*128..(mt+1)*128.
                            for mt in range(0, nh // 2, 2):
                                nc.tensor.matmul(
                                    ps,
                                    ctx_t[:, mt : mt + 2, m * 128 : (m + 1) * 128],
                                    wo_bf[:, mt : mt + 2, n0 : n0 + 512],
                                    start=(mt == 0),
                                    stop=(mt == nh // 2 - 2),
                                    perf_mode=DR,
                                )
                        pss.append((n0, ps))
                    xres = xres_tiles[m]
                    o = osb.tile([128, h], FP, name="osum")
                    for n0, ps in pss:
                        nc.vector.tensor_add(
                            out=o[:, n0 : n0 + 512], in0=ps, in1=xres[:, n0 : n0 + 512]
                        )
                    if "bo" in bcast:
                        nc.vector.tensor_add(out=o, in0=o, in1=bcast["bo"])
                    nsub = (h + 511) // 512
                    stats = lnp.tile([128, nsub, 6], FP, name="stats")
                    for i in range(nsub):
                        nc.vector.bn_stats(
                            out=stats[:, i, :], in_=o[:, i * 512 : (i + 1) * 512]
                        )
                    mv = lnp.tile([128, 2], FP, name="mv")
                    nc.vector.bn_aggr(out=mv, in_=stats)
                    std = lnp.tile([128, 1], FP, name="std")
                    nc.scalar.activation(std, mv[:, 1:2], AF.Sqrt, bias=eps_sb)
                    inv = lnp.tile([128, 1], FP, name="inv")
                    nc.vector.reciprocal(inv, std)
                    nb = lnp.tile([128, 1], FP, name="nb")
                    nc.vector.tensor_tensor(
                        out=nb, in0=mv[:, 0:1], in1=inv, op=OP.mult
                    )
                    nc.vector.tensor_scalar_mul(out=nb, in0=nb, scalar1=-1.0)
                    y = osb.tile([128, h], FP, name="yout")
                    nc.scalar.activation(
                        out=y, in_=o, func=AF.Identity, bias=nb, scale=inv
                    )
                    if "ln_gamma" in bcast:
                        nc.vector.tensor_mul(out=y, in0=y, in1=bcast["ln_gamma"])
                    if "ln_beta" in bcast:
                        nc.vector.tensor_add(out=y, in0=y, in1=bcast["ln_beta"])
                    nc.sync.dma_start(out=out[m * 128 : (m + 1) * 128, :], in_=y)

    _split_sync_waits(nc)
    return nc


_NC_CACHE = {}


def _get_nc(s, h, nh, sh, flags):
    key = (s, h, nh, sh, tuple(sorted(flags.items())))
    if key not in _NC_CACHE:
        _NC_CACHE[key] = _build(s, h, nh, sh, flags)
    return _NC_CACHE[key]


def _prepare(hidden_states, attention_mask, Wq, bq, Wk, bk, Wv, bv, Wo, bo, ln_gamma, ln_beta):
    hs = np.ascontiguousarray(np.asarray(hidden_states, dtype=np.float32))
    b_, s_, h_ = hs.shape
    nh_ = h_ // 64
    sh_ = s_ // 2
    am = np.asarray(attention_mask, dtype=np.float32).reshape(b_, s_)
    flags = {
        "bq": bool(np.any(np.asarray(bq))),
        "bk": bool(np.any(np.asarray(bk))),
        "bv": bool(np.any(np.asarray(bv))),
        "bo": bool(np.any(np.asarray(bo))),
        "ln_gamma": not bool(np.all(np.asarray(ln_gamma) == 1.0)),
        "ln_beta": bool(np.any(np.asarray(ln_beta))),
    }
    nc = _get_nc(s_, h_, nh_, sh_, flags)

    f32c = lambda a: np.ascontiguousarray(np.asarray(a, dtype=np.float32))
    f8c = lambda a, sc: np.ascontiguousarray(
        (np.asarray(a, dtype=np.float32) * sc).astype(ml_dtypes.float8_e4m3fn)
    )
    # weights x16 in fp8 (keeps small values out of the subnormal range);
    # K/Q both carry x16 so scores carry x256, folded into the Exp scale.
    # ctx_t carries x64 (x16 from V, x4 from the sum eviction), Wo x16, so
    # the out-proj PSUM carries x1024 — matched by scaling the residual
    # x1024 on the host. LayerNorm is scale-invariant, so the output is
    # unchanged.
    shared = {
        "wq": f8c(Wq, 16.0),
        "wk": f8c(Wk, 16.0),
        "wv": f8c(Wv, 64.0),
        "wo": f8c(Wo, 16.0),
    }
    scales = {"bq": 16.0, "bk": 16.0, "bv": 64.0, "bo": 1024.0}
    for name, arr in (
        ("bq", bq),
        ("bk", bk),
        ("bv", bv),
        ("bo", bo),
        ("ln_gamma", ln_gamma),
        ("ln_beta", ln_beta),
    ):
        if flags[name]:
            shared[name] = f32c(np.asarray(arr) * scales.get(name, 1.0))

    in_maps = []
    for c in range(N_CORES):
        bb, half = c // 2, c % 2
        mine = slice(half * sh_, (half + 1) * sh_)
        other = slice((1 - half) * sh_, (2 - half) * sh_)
        xp = np.concatenate([hs[bb, mine], hs[bb, other]], axis=0)
        xt = np.ascontiguousarray(xp.T.astype(ml_dtypes.float8_e4m3fn))
        mp = np.ascontiguousarray(np.concatenate([am[bb, mine], am[bb, other]]))
        in_maps.append(
            {
                "x": np.ascontiguousarray(xp[:sh_] * 1024.0),
                "xt": xt,
                "mask": mp,
                **shared,
            }
        )
    return nc, in_maps, (b_, s_, h_, sh_)


def _assemble(results, shape):
    b_, s_, h_, sh_ = shape
    out = np.empty((b_, s_, h_), dtype=np.float32)
    for c in range(N_CORES):
        bb, half = c // 2, c % 2
        out[bb, half * sh_ : (half + 1) * sh_] = results[c]["out"]
    return out


def kernel(**inputs) -> np.ndarray:
    nc, in_maps, shape = _prepare(**inputs)
    res = run_bass_kernel_spmd(nc, in_maps, core_ids=list(range(N_CORES)))
    return _assemble(res.results, shape)
